# revision 2
# baseline (speedup 1.0000x reference)
"""Multi-head attention (B=2, N=4096, C=512, H=8) on 8 TRN2 NeuronCores.

Sharding: core c handles batch c//4 and heads {2*(c%4), 2*(c%4)+1}
(data parallel over batch, tensor parallel over heads). Each core
computes its 2 heads' attention plus a partial output projection;
the host sums the 4 partials per batch and adds the bias terms
(b_out and b_v @ W_out, which commutes past softmax-weighted sums).

v2 changes vs baseline (engine-balance: ACT and PE were both ~84%):
  q projection   ONE stacked M=128 matmul per block (both heads);
                 the per-head zero-padded qT halves are produced by a
                 DVE mask-mult+bias (same DVE cost as before, half the
                 PE cost)
  out projection oT holds h0 dims on partitions 0:64 and h1 on
                 64:128, so ONE K=128 matmul per 128-token chunk
                 replaces two zero-padded ones.  Both heads' PV
                 accumulators stay at PSUM base 0 ([v|1], sums at
                 64); the h1 normalize writes cross-partition-base
                 (in at 0:64, out at 64:128), which DVE APs support
                 (the baseline's reciprocal already relied on it).
  exp offload    a subset of score groups is exponentiated on DVE via
                 the Schraudolph bit trick (bits16 = round(A*s + B)
                 reinterpreted as bf16; ~3% max rel err on those
                 groups, verified 5e-3 end-to-end) to pull ACT below
                 the PE roofline
  ACT staging    PSUM->SBUF staging copies (v production, out
                 staging, sums broadcast) ride the ACT engine's
                 headroom (Pool cannot access PSUM)
"""

import numpy as np

import concourse.bass as bass
import concourse.mybir as mybir
import concourse.tile as tile
from concourse.bass_utils import run_bass_kernel_spmd
from concourse.tile_rust import add_dep_helper
from concourse.vector_clock import ScopedClock

F32 = mybir.dt.float32
F32R = mybir.dt.float32r
BF16 = mybir.dt.bfloat16
I16 = mybir.dt.int16
AF = mybir.ActivationFunctionType
ALU = mybir.AluOpType

B, N, C, H = 2, 4096, 512, 8
HD = C // H          # 64
HPC = H // 4         # 2 heads per core
NCORES = 8
NT = N // 128        # 32 key chunks
NCJ = C // 128       # 4 contraction chunks
QB = N // 512        # 8 query blocks
GP = 2               # key chunks per exp group (PSUM banks per scores tile)
NG = NT // GP
NUNITS = QB * HPC
VW = 2 * (HD + 1)    # v_nat row width: [v0|1][v1|1] (ones via memset)
SCALE = 1.0 / float(np.sqrt(C))
# Schraudolph bf16 exp: bits16 = round(SCH_A * score + SCH_B) viewed as bf16
SCH_A = 128.0 / float(np.log(2.0)) * SCALE
SCH_B = 16250.5
# odd groups exponentiate on DVE, even on ACT: the two engines run
# concurrently so the exp stream (~1.1us/group on one engine) stops
# pacing the 854ns/group PE pipeline
SCH_GROUPS = frozenset(range(1, NG, 2))


def _patch_tail_drain():
    """This walrus build caps sync waits at 1 per non-EventSemaphore
    instruction (2 for EventSemaphore); the stock TileContext tail-drain
    attaches every outstanding wait to one Drain, and the scheduler can
    leave >1 wait on regular instructions. Spill extras onto fresh
    same-engine nops inserted just before the over-subscribed one."""
    if getattr(tile.TileContext, "_drain_patched", False):
        return

    def _spill_excess_waits(nc):
        for fn in nc.m.functions:
            for bb in fn.blocks:
                insts = bb.instructions
                i = 0
                while i < len(insts):
                    inst = insts[i]
                    si = inst.sync_info
                    cap = 2 if isinstance(inst, mybir.InstEventSemaphore) else 1
                    if si is None or len(si.on_wait) <= cap:
                        i += 1
                        continue
                    extra = list(si.on_wait[cap:])
                    si.on_wait[:] = si.on_wait[:cap]
                    for w in extra:
                        nop = nc.engines[inst.engine].nop(
                            hint="wait_spill", nofuse=True
                        )
                        cur = nc.cur_bb.bb.instructions
                        cur.remove(nop.ins)
                        if nop.ins.sync_info is None:
                            nop.ins.sync_info = mybir.SyncInfo(
                                on_update=[], on_wait=[]
                            )
                        nop.ins.sync_info.on_wait.append(w)
                        insts.insert(i, nop.ins)
                        i += 1
                    i += 1

    def _drain_and_barrier(self, tick_clock, wait_clock):
        nc = self.nc
        drain_inst = nc.sync.drain()
        wait_clock.add_sem_waits(
            drain_inst.ins, ScopedClock({None: tick_clock.global_clock})
        )
        nc.all_engine_barrier()
        assert self.sems is not None
        popped = nc._tile_sem_poison_stack.pop()
        assert popped is self._sem_poison
        nc.clear_and_free_semaphores(list(self.sems.allocated().values()))
        nc.all_engine_barrier()
        _spill_excess_waits(nc)

    tile.TileContext._drain_and_barrier = _drain_and_barrier
    tile.TileContext._drain_patched = True


def _build_program():
    _patch_tail_drain()
    nc = bass.Bass()

    xt = nc.dram_tensor("xt", [QB, 128, NCJ, 512], BF16, kind="ExternalInput")
    # host-prearranged weight layouts (see kernel() below)
    w_q = nc.dram_tensor("w_q", [128, NCJ, 128], BF16, kind="ExternalInput")
    w_k = nc.dram_tensor("w_k", [128, NCJ, 128], BF16, kind="ExternalInput")
    # v weights carry zero cols at 64 and 129 so one copy + tiny memsets
    # yield the augmented [v0|1][v1|1] PV-operand layout
    w_v = nc.dram_tensor("w_v", [128, NCJ, VW], BF16, kind="ExternalInput")
    w_o = nc.dram_tensor("w_o", [128, C], BF16, kind="ExternalInput")
    b_qm = nc.dram_tensor("b_qm", [128, HPC], F32, kind="ExternalInput")
    qmask = nc.dram_tensor("qmask", [128, HPC], F32, kind="ExternalInput")
    b_k = nc.dram_tensor("b_k", [128, 1], F32, kind="ExternalInput")
    ones = nc.dram_tensor("ones", [1, HD], F32R, kind="ExternalInput")
    out = nc.dram_tensor("out", [N, C], F32, kind="ExternalOutput")

    from contextlib import ExitStack

    with tile.TileContext(nc) as tc, ExitStack() as ctx:
        const = ctx.enter_context(tc.tile_pool(name="const", bufs=1))
        w_q_sb = const.tile([128, NCJ, 128], BF16)
        w_k_sb = const.tile([128, NCJ, 128], BF16)
        w_v_sb = const.tile([128, NCJ, VW], BF16)
        w_o_sb = const.tile([128, C], BF16)
        b_qm_sb = const.tile([128, HPC], F32)
        qmask_sb = const.tile([128, HPC], F32)
        b_k_sb = const.tile([128, 1], F32)
        ones_sb = const.tile([1, HD], F32R)


        persist = ctx.enter_context(tc.tile_pool(name="persist", bufs=1))
        # K=128 zero-padded: per head, the dead 64 rows stay zero (via the
        # qmask mult) -- K=64 M=128 single-group matmuls run at half rate
        qT = persist.tile([128, HPC, N], BF16)
        # kT holds BOTH heads stacked (rows 0:64 = h0, 64:128 = h1); the
        # per-head qT zero rows select the head in the scores contraction
        kT = persist.tile([128, N], BF16)
        # [tokens, kb, VW]: [v0 dims | 1 | 1 | v1 dims] per key chunk
        v_nat = persist.tile([128, NT, VW], BF16)

        # ---- fused pipeline: qkv production interleaved into attention ----
        with (
            tc.tile_pool(name="xTp", bufs=1) as xTp,
            tc.tile_pool(name="oTp", bufs=1) as oTp,
            tc.tile_pool(name="expp", bufs=6) as expp,
            tc.tile_pool(name="recipp", bufs=3) as recipp,
            tc.tile_pool(name="bcsb", bufs=3) as bcsb,
            tc.tile_pool(name="ostage", bufs=4) as ostage,
            tc.tile_pool(name="ps_s", bufs=3, space="PSUM") as ps_s,
            tc.tile_pool(name="ps_o", bufs=2, space="PSUM") as ps_o,
        ):
            # production/projection PSUM tiles borrow score-pipeline slots
            # (tag-shared, temporally interleaved); 3x[128,2,512] score
            # slots + 2x[65,512] po = exactly 8 PSUM banks
            ps_p = ps_s
            # oT: h0 dims on partitions 0:64, h1 dims on 64:128 -- the
            # out-projection contracts both heads in one K=128 matmul
            oT = oTp.tile([128, N], BF16)
            xT = xTp.tile([128, NCJ, N], BF16)
            # block-0 x and the earliest-consumed weights first, so
            # production(0) isn't blocked behind the full weight load
            nc.sync.dma_start(out=xT[:, :, 0:512], in_=xt[0])
            nc.sync.dma_start(out=w_k_sb, in_=w_k[:])
            nc.sync.dma_start(out=b_k_sb, in_=b_k[:])
            nc.sync.dma_start(out=w_v_sb, in_=w_v[:])
            nc.sync.dma_start(out=w_q_sb, in_=w_q[:])
            nc.sync.dma_start(out=b_qm_sb, in_=b_qm[:])
            nc.sync.dma_start(out=qmask_sb, in_=qmask[:])
            nc.sync.dma_start(out=ones_sb, in_=ones[:])
            for tb in range(1, QB):
                tsl = slice(tb * 512, (tb + 1) * 512)
                nc.sync.dma_start(out=xT[:, :, tsl], in_=xt[tb])
            nc.sync.dma_start(out=w_o_sb, in_=w_o[:])

            def vnat_kb(kb):
                ksl = slice(kb * 128, (kb + 1) * 128)
                pv_ = ps_p.tile([128, VW], F32, tag="ks", name="pv_")
                for cj in range(NCJ):
                    nc.tensor.matmul(
                        pv_,
                        lhsT=xT[:, cj, ksl],
                        rhs=w_v_sb[:, cj, :],
                        start=(cj == 0),
                        stop=(cj == NCJ - 1),
                    )
                nc.scalar.activation(
                    out=v_nat[:, kb, :], in_=pv_, func=AF.Copy
                )
                # restore the softmax-sum ones over the zero cols
                nc.gpsimd.memset(v_nat[:, kb, HD:HD + 1], 1.0)
                nc.gpsimd.memset(v_nat[:, kb, VW - 1:VW], 1.0)

            def production(tb, part=None):
                """qkv projections for one 512-token block; psum via the
                shared 'pp' tag (temporally disjoint from proj use).
                part 0: k (scores consume it soonest) + first half of v;
                part 1: rest of v + q. None: both."""
                tsl = slice(tb * 512, (tb + 1) * 512)
                if part in (0, None):
                    pk = ps_p.tile([128, 512], F32, tag="ks", name="pk")
                    for cj in range(NCJ):
                        nc.tensor.matmul(
                            pk,
                            lhsT=w_k_sb[:, cj, :],
                            rhs=xT[:, cj, tsl],
                            start=(cj == 0),
                            stop=(cj == NCJ - 1),
                        )
                    nc.vector.tensor_scalar_add(
                        out=kT[:, tsl], in0=pk, scalar1=b_k_sb
                    )
                    for kb in range(tb * 4, tb * 4 + 2):
                        vnat_kb(kb)
                if part in (1, None):
                    for kb in range(tb * 4 + 2, tb * 4 + 4):
                        vnat_kb(kb)
                    # ONE stacked q matmul; per-head halves split out by a
                    # DVE mask-mult (dead rows -> 0) + masked-bias add
                    pm = ps_p.tile([128, 512], F32, tag="ks", name="pm")
                    for cj in range(NCJ):
                        nc.tensor.matmul(
                            pm,
                            lhsT=w_q_sb[:, cj, :],
                            rhs=xT[:, cj, tsl],
                            start=(cj == 0),
                            stop=(cj == NCJ - 1),
                        )
                    for h in range(HPC):
                        nc.vector.tensor_scalar(
                            out=qT[:, h, tsl],
                            in0=pm,
                            scalar1=qmask_sb[:, h:h + 1],
                            scalar2=b_qm_sb[:, h:h + 1],
                            op0=ALU.mult,
                            op1=ALU.add,
                        )

            production(0)
            last_exp = {"inst": None}
            pending_recip = []  # flushed @g1 of the following unit (DVE)
            pending_bc = []     # flushed @g8 (Pool bcast DMA + DVE normalize)
            pending_proj = []   # flushed @g9/11/13/15 (PE matmuls)

            def make_tail(po, h, qsl, u):
                state = {}
                prow = slice(0, HD) if h == 0 else slice(HD, 128)

                def recip():
                    rt = recipp.tile([1, 512], F32R, name="rt")
                    with nc.allow_low_precision(
                        reason="f32r reciprocal feeds the f32r "
                        "broadcast matmul (19-bit mantissa ample)"
                    ):
                        nc.vector.reciprocal(out=rt, in_=po[HD:HD + 1, :])
                    state["rt"] = rt

                def bcmult():
                    # broadcast 1/sums across 64 partitions with a K=1
                    # ones-matmul (f32r streams 512 cols at full rate) --
                    # cheaper and far lower latency than a DRAM round-trip.
                    # Staged through SBUF: tensor_tensor allows only one
                    # PSUM operand (po), so bc must be SBUF-resident.
                    bc = ps_s.tile([HD, 512], F32, tag="ks", name="bc")
                    nc.tensor.matmul(
                        bc,
                        lhsT=ones_sb,
                        rhs=state["rt"],
                        start=True,
                        stop=True,
                    )
                    bcs = bcsb.tile([HD, 512], F32, name="bcs")
                    nc.scalar.activation(out=bcs, in_=bc, func=AF.Copy)
                    nc.vector.tensor_mul(
                        out=oT[prow, qsl], in0=po[0:HD, :], in1=bcs
                    )
                return recip, bcmult

            def make_proj(qb, j):
                def proj():
                    q0 = qb * 512 + j * 128
                    pp = ps_p.tile([128, C], F32, tag="ks", name="pp")
                    mm = nc.tensor.matmul(
                        pp,
                        lhsT=oT[:, q0:q0 + 128],
                        rhs=w_o_sb,
                        start=True,
                        stop=True,
                    )
                    if last_exp["inst"] is not None:
                        add_dep_helper(
                            mm.ins, last_exp["inst"], sync=False,
                            reason="proj after normalize really done",
                        )
                    ot = ostage.tile([128, C], F32, name="ot")
                    nc.scalar.activation(out=ot, in_=pp, func=AF.Copy)
                    nc.sync.dma_start(out=out[q0:q0 + 128, :], in_=ot)
                return proj

            units = [(qb, h) for qb in range(QB) for h in range(HPC)]

            def s_group(u, g):
                qb, h = units[u]
                qsl = slice(qb * 512, (qb + 1) * 512)
                ks = ps_s.tile([128, GP, 512], F32, tag="ks", name="ks")
                for j in range(GP):
                    kb = g * GP + j
                    nc.tensor.matmul(
                        ks[:, j, :],
                        lhsT=kT[:, kb * 128:(kb + 1) * 128],
                        rhs=qT[:, h, qsl],
                        start=True,
                        stop=True,
                    )
                return ks

            # flat (unit, group) pipeline: the scores skew carries across
            # unit boundaries so the PE/ACT streams never drain
            flat = [(u, g) for u in range(len(units)) for g in range(NG)]
            po_tiles = {}
            pend = s_group(*flat[0])
            for i, (u, g) in enumerate(flat):
                ks = pend
                pend = s_group(*flat[i + 1]) if i + 1 < len(flat) else None
                if u == 0 and g // 2 + 1 < QB:
                    production(g // 2 + 1, part=g % 2)
                if g == 1:
                    for f in pending_recip:
                        f()
                    pending_recip.clear()
                elif g == 8:
                    for f in pending_bc:
                        f()
                    pending_bc.clear()
                elif g in (9, 11, 13, 15) and pending_proj:
                    pending_proj.pop(0)()
                et = expp.tile([128, GP, 512], BF16)
                if g in SCH_GROUPS:
                    # Schraudolph: bf16 bits of exp(SCALE*s) via one DVE
                    # affine op, int16-round into the bf16 tile
                    exp_bi = nc.vector.tensor_scalar(
                        out=et[:, :, :].bitcast(I16),
                        in0=ks,
                        scalar1=SCH_A,
                        scalar2=SCH_B,
                        op0=ALU.mult,
                        op1=ALU.add,
                    )
                else:
                    exp_bi = nc.scalar.activation(
                        out=et, in_=ks, func=AF.Exp, scale=SCALE
                    )
                last_exp["inst"] = exp_bi.ins
                qb, h = units[u]
                if g == 0:
                    po_tiles[u] = ps_o.tile([HD + 1, 512], F32, name="po")
                po = po_tiles[u]
                vsl = slice(0, HD + 1) if h == 0 else slice(HD + 1, VW)
                for j in range(GP):
                    kb = g * GP + j
                    nc.tensor.matmul(
                        po,
                        lhsT=v_nat[:, kb, vsl],
                        rhs=et[:, j, :],
                        start=(kb == 0),
                        stop=(kb == NT - 1),
                    )
                if g == NG - 1:
                    qsl = slice(qb * 512, (qb + 1) * 512)
                    recip, bcmult = make_tail(po_tiles.pop(u), h, qsl, u)
                    pending_recip.append(recip)
                    pending_bc.append(bcmult)
                    if h == HPC - 1:
                        for j in range(4):
                            pending_proj.append(make_proj(qb, j))
            for f in pending_recip:
                f()
            for f in pending_bc:
                f()
            for f in pending_proj:
                f()

    return nc


_PROGRAM = None


def _get_program():
    global _PROGRAM
    if _PROGRAM is None:
        _PROGRAM = _build_program()
    return _PROGRAM


def _bf16(a):
    import ml_dtypes

    return np.asarray(a, dtype=np.float32).astype(ml_dtypes.bfloat16)


def _prep_core_inputs(x, W_qkv, b_qkv, heads, batch):
    """Host-side slicing/relayout for one core."""
    cols = np.concatenate([np.arange(h * HD, (h + 1) * HD) for h in heads])
    w_q = W_qkv[:, cols]               # [512, 128]
    w_k = W_qkv[:, C + cols]
    w_v = W_qkv[:, 2 * C + cols]
    # q and k: both heads stacked in one M=128 projection
    w_q = np.ascontiguousarray(w_q.reshape(NCJ, 128, 128).transpose(1, 0, 2))
    w_k = np.ascontiguousarray(w_k.reshape(NCJ, 128, 128).transpose(1, 0, 2))
    # v: [v0 | 0 | v1 | 0] with zero cols where the ones columns go
    wv4 = w_v.reshape(NCJ, 128, HPC, HD)
    w_vp = np.zeros((NCJ, 128, VW), dtype=np.float32)
    w_vp[:, :, 0:HD] = wv4[:, :, 0, :]
    w_vp[:, :, HD + 1:VW - 1] = wv4[:, :, 1, :]
    w_v = np.ascontiguousarray(w_vp.transpose(1, 0, 2))

    # per-head live-row masks and masked biases for the qT split
    qmask = np.zeros((128, HPC), dtype=np.float32)
    b_qm = np.zeros((128, HPC), dtype=np.float32)
    bq = b_qkv[cols]
    for h in range(HPC):
        qmask[h * HD:(h + 1) * HD, h] = 1.0
        b_qm[h * HD:(h + 1) * HD, h] = bq[h * HD:(h + 1) * HD]
    b_k = b_qkv[C + cols].reshape(128, 1).astype(np.float32)
    xt = np.ascontiguousarray(
        x[batch].T.reshape(NCJ, 128, QB, 512).transpose(2, 1, 0, 3))
    return {
        "ones": np.ones((1, HD), dtype=np.float32),
        "xt": _bf16(xt),
        "w_q": _bf16(w_q),
        "w_k": _bf16(w_k),
        "w_v": _bf16(w_v),
        "b_qm": b_qm,
        "qmask": qmask,
        "b_k": b_k,
    }


def _core_w_o(W_out, heads):
    rows = np.concatenate([np.arange(h * HD, (h + 1) * HD) for h in heads])
    return _bf16(W_out[rows])


def kernel(x, W_qkv, b_qkv, W_out, b_out):
    x = np.asarray(x, dtype=np.float32)
    W_qkv = np.asarray(W_qkv, dtype=np.float32)
    b_qkv = np.asarray(b_qkv, dtype=np.float32)
    W_out = np.asarray(W_out, dtype=np.float32)
    b_out = np.asarray(b_out, dtype=np.float32)

    nc = _get_program()
    in_maps = []
    for c in range(NCORES):
        batch, hp = c // 4, c % 4
        heads = [2 * hp, 2 * hp + 1]
        im = _prep_core_inputs(x, W_qkv, b_qkv, heads, batch)
        im["w_o"] = _core_w_o(W_out, heads)
        in_maps.append(im)

    res = run_bass_kernel_spmd(nc, in_maps, core_ids=list(range(NCORES)))

    # v-bias commutes: softmax rows sum to 1, so (P @ (V + 1 b_v)) @ W_o
    # = P@V@W_o + b_v@W_o. Add b_v@W_out and b_out once on the host.
    const_row = b_qkv[2 * C:] @ W_out + b_out    # [512]
    out = np.empty((B, N, C), dtype=np.float32)
    for b in range(B):
        acc = res.results[4 * b]["out"].astype(np.float32).copy()
        for c in range(4 * b + 1, 4 * b + 4):
            acc += res.results[c]["out"]
        out[b] = acc + const_row
    return out


# revision 3
# speedup vs baseline: 1.2229x; 1.2229x over previous
"""Multi-head attention (B=2, N=4096, C=512, H=8) on 8 TRN2 NeuronCores.

Sharding: core c handles batch c//4 and heads {2*(c%4), 2*(c%4)+1}
(data parallel over batch, tensor parallel over heads). Each core
computes its 2 heads' attention plus a partial output projection;
the host sums the 4 partials per batch and adds the bias terms
(b_out and b_v @ W_out, which commutes past softmax-weighted sums).

v2 changes vs baseline (engine-balance: ACT and PE were both ~84%):
  q projection   ONE stacked M=128 matmul per block (both heads);
                 the per-head zero-padded qT halves are produced by a
                 DVE mask-mult+bias (same DVE cost as before, half the
                 PE cost)
  out projection oT holds h0 dims on partitions 0:64 and h1 on
                 64:128, so ONE K=128 matmul per 128-token chunk
                 replaces two zero-padded ones.  Both heads' PV
                 accumulators stay at PSUM base 0 ([v|1], sums at
                 64); the h1 normalize writes cross-partition-base
                 (in at 0:64, out at 64:128), which DVE APs support
                 (the baseline's reciprocal already relied on it).
  exp offload    a subset of score groups is exponentiated on DVE via
                 the Schraudolph bit trick (bits16 = round(A*s + B)
                 reinterpreted as bf16; ~3% max rel err on those
                 groups, verified 5e-3 end-to-end) to pull ACT below
                 the PE roofline
  ACT staging    PSUM->SBUF staging copies (v production, out
                 staging, sums broadcast) ride the ACT engine's
                 headroom (Pool cannot access PSUM)
"""

import numpy as np

import concourse.bass as bass
import concourse.mybir as mybir
import concourse.tile as tile
from concourse.bass_utils import run_bass_kernel_spmd
from concourse.tile_rust import add_dep_helper
from concourse.vector_clock import ScopedClock

F32 = mybir.dt.float32
F32R = mybir.dt.float32r
BF16 = mybir.dt.bfloat16
FP8 = mybir.dt.float8e4
I16 = mybir.dt.int16
I8 = mybir.dt.int8
AF = mybir.ActivationFunctionType
ALU = mybir.AluOpType

B, N, C, H = 2, 4096, 512, 8
HD = C // H          # 64
HPC = H // 4         # 2 heads per core
NCORES = 8
NT = N // 128        # 32 key chunks
NCJ = C // 128       # 4 contraction chunks
QB = N // 512        # 8 query blocks
GP = 2               # key chunks per exp group (PSUM banks per scores tile)
NG = NT // GP
NUNITS = QB * HPC
VW = 2 * (HD + 1)    # v_nat row width: [v0|1][v1|1] (ones via memset)
SCALE = 1.0 / float(np.sqrt(C))
# Schraudolph bf16 exp: bits16 = round(SCH_A * score + SCH_B) viewed as bf16
SCH_A = 128.0 / float(np.log(2.0)) * SCALE
SCH_B = 16250.5
# fp8(e4m3) variant feeding the DoubleRow PV matmul
SCH8_A = 8.0 / float(np.log(2.0)) * SCALE
SCH8_B = 55.625
VP8 = 80             # fp8 v_nat per-head row pitch (16B-aligned for DoubleRow)
# odd groups exponentiate on DVE, even on ACT: the two engines run
# concurrently so the exp stream (~1.1us/group on one engine) stops
# pacing the 854ns/group PE pipeline
SCH_GROUPS = frozenset(range(1, NG, 2))


def _patch_tail_drain():
    """This walrus build caps sync waits at 1 per non-EventSemaphore
    instruction (2 for EventSemaphore); the stock TileContext tail-drain
    attaches every outstanding wait to one Drain, and the scheduler can
    leave >1 wait on regular instructions. Spill extras onto fresh
    same-engine nops inserted just before the over-subscribed one."""
    if getattr(tile.TileContext, "_drain_patched", False):
        return

    def _spill_excess_waits(nc):
        for fn in nc.m.functions:
            for bb in fn.blocks:
                insts = bb.instructions
                i = 0
                while i < len(insts):
                    inst = insts[i]
                    si = inst.sync_info
                    cap = 2 if isinstance(inst, mybir.InstEventSemaphore) else 1
                    if si is None or len(si.on_wait) <= cap:
                        i += 1
                        continue
                    extra = list(si.on_wait[cap:])
                    si.on_wait[:] = si.on_wait[:cap]
                    for w in extra:
                        nop = nc.engines[inst.engine].nop(
                            hint="wait_spill", nofuse=True
                        )
                        cur = nc.cur_bb.bb.instructions
                        cur.remove(nop.ins)
                        if nop.ins.sync_info is None:
                            nop.ins.sync_info = mybir.SyncInfo(
                                on_update=[], on_wait=[]
                            )
                        nop.ins.sync_info.on_wait.append(w)
                        insts.insert(i, nop.ins)
                        i += 1
                    i += 1

    def _drain_and_barrier(self, tick_clock, wait_clock):
        nc = self.nc
        drain_inst = nc.sync.drain()
        wait_clock.add_sem_waits(
            drain_inst.ins, ScopedClock({None: tick_clock.global_clock})
        )
        nc.all_engine_barrier()
        assert self.sems is not None
        popped = nc._tile_sem_poison_stack.pop()
        assert popped is self._sem_poison
        nc.clear_and_free_semaphores(list(self.sems.allocated().values()))
        nc.all_engine_barrier()
        _spill_excess_waits(nc)

    tile.TileContext._drain_and_barrier = _drain_and_barrier
    tile.TileContext._drain_patched = True


def _build_program():
    _patch_tail_drain()
    nc = bass.Bass()

    xt = nc.dram_tensor("xt", [QB, 128, NCJ, 512], BF16, kind="ExternalInput")
    # host-prearranged weight layouts (see kernel() below)
    w_q = nc.dram_tensor("w_q", [128, NCJ, 128], BF16, kind="ExternalInput")
    w_k = nc.dram_tensor("w_k", [128, NCJ, 128], BF16, kind="ExternalInput")
    # v weights carry zero cols at 64 and 129 so one copy + tiny memsets
    # yield the augmented [v0|1][v1|1] PV-operand layout
    w_v = nc.dram_tensor("w_v", [128, NCJ, VW], BF16, kind="ExternalInput")
    w_o = nc.dram_tensor("w_o", [128, C], BF16, kind="ExternalInput")
    b_qm = nc.dram_tensor("b_qm", [128, HPC], F32, kind="ExternalInput")
    qmask = nc.dram_tensor("qmask", [128, HPC], F32, kind="ExternalInput")
    b_k = nc.dram_tensor("b_k", [128, 1], F32, kind="ExternalInput")
    ones = nc.dram_tensor("ones", [1, HD], F32R, kind="ExternalInput")
    out = nc.dram_tensor("out", [N, C], F32, kind="ExternalOutput")

    from contextlib import ExitStack

    with tile.TileContext(nc) as tc, ExitStack() as ctx:
        const = ctx.enter_context(tc.tile_pool(name="const", bufs=1))
        w_q_sb = const.tile([128, NCJ, 128], BF16)
        w_k_sb = const.tile([128, NCJ, 128], BF16)
        w_v_sb = const.tile([128, NCJ, VW], BF16)
        w_o_sb = const.tile([128, C], BF16)
        b_qm_sb = const.tile([128, HPC], F32)
        qmask_sb = const.tile([128, HPC], F32)
        b_k_sb = const.tile([128, 1], F32)
        ones_sb = const.tile([1, HD], F32R)


        persist = ctx.enter_context(tc.tile_pool(name="persist", bufs=1))
        # K=128 zero-padded: per head, the dead 64 rows stay zero (via the
        # qmask mult) -- K=64 M=128 single-group matmuls run at half rate
        qT = persist.tile([128, HPC, N], BF16)
        # kT holds BOTH heads stacked (rows 0:64 = h0, 64:128 = h1); the
        # per-head qT zero rows select the head in the scores contraction
        kT = persist.tile([128, N], BF16)
        # fp8 [tokens, kb, head, VP8]: [v dims | 1 | pad] per key chunk --
        # lhsT of the DoubleRow PV matmul ([128, 2, 65] per chunk pair)
        v_nat = persist.tile([128, NT, HPC, VP8], FP8)

        # ---- fused pipeline: qkv production interleaved into attention ----
        with (
            tc.tile_pool(name="xTp", bufs=1) as xTp,
            tc.tile_pool(name="oTp", bufs=1) as oTp,
            tc.tile_pool(name="expp", bufs=6) as expp,
            tc.tile_pool(name="recipp", bufs=3) as recipp,
            tc.tile_pool(name="bcsb", bufs=3) as bcsb,
            tc.tile_pool(name="ostage", bufs=4) as ostage,
            tc.tile_pool(name="ps_s", bufs=3, space="PSUM") as ps_s,
            tc.tile_pool(name="ps_o", bufs=2, space="PSUM") as ps_o,
        ):
            # production/projection PSUM tiles borrow score-pipeline slots
            # (tag-shared, temporally interleaved); 3x[128,2,512] score
            # slots + 2x[65,512] po = exactly 8 PSUM banks
            ps_p = ps_s
            # oT: h0 dims on partitions 0:64, h1 dims on 64:128 -- the
            # out-projection contracts both heads in one K=128 matmul
            oT = oTp.tile([128, N], BF16)
            xT = xTp.tile([128, NCJ, N], BF16)
            # block-0 x and the earliest-consumed weights first, so
            # production(0) isn't blocked behind the full weight load;
            # split per cj across queues so the four DMAs run in parallel
            for cj, eng in enumerate((nc.sync, nc.scalar, nc.gpsimd,
                                      nc.sync)):
                eng.dma_start(out=xT[:, cj, 0:512], in_=xt[0, :, cj])
            nc.sync.dma_start(out=w_k_sb, in_=w_k[:])
            nc.sync.dma_start(out=b_k_sb, in_=b_k[:])
            nc.sync.dma_start(out=w_v_sb, in_=w_v[:])
            nc.sync.dma_start(out=w_q_sb, in_=w_q[:])
            nc.sync.dma_start(out=b_qm_sb, in_=b_qm[:])
            nc.sync.dma_start(out=qmask_sb, in_=qmask[:])
            nc.sync.dma_start(out=ones_sb, in_=ones[:])
            for tb in range(1, QB):
                tsl = slice(tb * 512, (tb + 1) * 512)
                nc.sync.dma_start(out=xT[:, :, tsl], in_=xt[tb])
            nc.sync.dma_start(out=w_o_sb, in_=w_o[:])

            def vnat_kb(kb):
                ksl = slice(kb * 128, (kb + 1) * 128)
                pv_ = ps_p.tile([128, VW], F32, tag="ks", name="pv_")
                for cj in range(NCJ):
                    nc.tensor.matmul(
                        pv_,
                        lhsT=xT[:, cj, ksl],
                        rhs=w_v_sb[:, cj, :],
                        start=(cj == 0),
                        stop=(cj == NCJ - 1),
                    )
                nc.scalar.activation(
                    out=v_nat[:, kb, :, 0:HD + 1],
                    in_=pv_.rearrange("p (h d) -> p h d", h=HPC),
                    func=AF.Copy,
                )
                # restore the softmax-sum ones over the zero cols
                nc.gpsimd.memset(v_nat[:, kb, :, HD:HD + 1], 1.0)

            def production(tb, part=None):
                """qkv projections for one 512-token block; psum via the
                shared 'pp' tag (temporally disjoint from proj use).
                part 0: k (scores consume it soonest) + first half of v;
                part 1: rest of v + q. None: both."""
                tsl = slice(tb * 512, (tb + 1) * 512)
                if part in (0, None):
                    pk = ps_p.tile([128, 512], F32, tag="ks", name="pk")
                    for cj in range(NCJ):
                        nc.tensor.matmul(
                            pk,
                            lhsT=w_k_sb[:, cj, :],
                            rhs=xT[:, cj, tsl],
                            start=(cj == 0),
                            stop=(cj == NCJ - 1),
                        )
                    nc.scalar.activation(
                        out=kT[:, tsl], in_=pk, func=AF.Identity,
                        bias=b_k_sb[:, 0:1],
                    )
                    for kb in range(tb * 4, tb * 4 + 2):
                        vnat_kb(kb)
                if part in (1, None):
                    for kb in range(tb * 4 + 2, tb * 4 + 4):
                        vnat_kb(kb)
                    # ONE stacked q matmul; per-head halves split out by a
                    # DVE mask-mult (dead rows -> 0) + masked-bias add
                    pm = ps_p.tile([128, 512], F32, tag="ks", name="pm")
                    for cj in range(NCJ):
                        nc.tensor.matmul(
                            pm,
                            lhsT=w_q_sb[:, cj, :],
                            rhs=xT[:, cj, tsl],
                            start=(cj == 0),
                            stop=(cj == NCJ - 1),
                        )
                    nc.scalar.activation(
                        out=qT[:, 0, tsl], in_=pm, func=AF.Identity,
                        scale=qmask_sb[:, 0:1], bias=b_qm_sb[:, 0:1],
                    )
                    nc.vector.tensor_scalar(
                        out=qT[:, 1, tsl],
                        in0=pm,
                        scalar1=qmask_sb[:, 1:2],
                        scalar2=b_qm_sb[:, 1:2],
                        op0=ALU.mult,
                        op1=ALU.add,
                    )

            production(0)
            last_exp = {"inst": None}
            pending_recip = []  # flushed @g1 of the following unit (DVE)
            pending_bc = []     # flushed @g8 (Pool bcast DMA + DVE normalize)
            pending_proj = []   # flushed @g9/11/13/15 (PE matmuls)

            def make_tail(po, h, qsl, u):
                state = {}
                prow = slice(0, HD) if h == 0 else slice(HD, 128)

                def recip():
                    rt = recipp.tile([1, 512], F32R, name="rt")
                    with nc.allow_low_precision(
                        reason="f32r reciprocal feeds the f32r "
                        "broadcast matmul (19-bit mantissa ample)"
                    ):
                        nc.vector.reciprocal(out=rt, in_=po[HD:HD + 1, :])
                    state["rt"] = rt

                def bcmult():
                    # broadcast 1/sums across 64 partitions with a K=1
                    # ones-matmul (f32r streams 512 cols at full rate) --
                    # cheaper and far lower latency than a DRAM round-trip.
                    # Staged through SBUF: tensor_tensor allows only one
                    # PSUM operand (po), so bc must be SBUF-resident.
                    bc = ps_s.tile([HD, 512], F32, tag="ks", name="bc")
                    nc.tensor.matmul(
                        bc,
                        lhsT=ones_sb,
                        rhs=state["rt"],
                        start=True,
                        stop=True,
                    )
                    bcs = bcsb.tile([HD, 512], F32, name="bcs")
                    nc.scalar.activation(out=bcs, in_=bc, func=AF.Copy)
                    if u == NUNITS - 1:
                        # last unit: split the normalize so each half's
                        # projections can start as soon as it lands
                        for ci in range(2):
                            cs = slice(ci * 256, (ci + 1) * 256)
                            qs = slice(qsl.start + ci * 256,
                                       qsl.start + (ci + 1) * 256)
                            nc.vector.tensor_mul(
                                out=oT[prow, qs], in0=po[0:HD, cs],
                                in1=bcs[:, cs],
                            )
                    else:
                        nc.vector.tensor_mul(
                            out=oT[prow, qsl], in0=po[0:HD, :], in1=bcs
                        )
                return recip, bcmult

            def make_proj(qb, j):
                def proj():
                    q0 = qb * 512 + j * 128
                    pp = ps_p.tile([128, C], F32, tag="ks", name="pp")
                    mm = nc.tensor.matmul(
                        pp,
                        lhsT=oT[:, q0:q0 + 128],
                        rhs=w_o_sb,
                        start=True,
                        stop=True,
                    )
                    if last_exp["inst"] is not None:
                        add_dep_helper(
                            mm.ins, last_exp["inst"], sync=False,
                            reason="proj after normalize really done",
                        )
                    ot = ostage.tile([128, C], F32, name="ot")
                    nc.scalar.activation(out=ot, in_=pp, func=AF.Copy)
                    nc.sync.dma_start(out=out[q0:q0 + 128, :], in_=ot)
                return proj

            units = [(qb, h) for qb in range(QB) for h in range(HPC)]

            def s_group(u, g):
                qb, h = units[u]
                qsl = slice(qb * 512, (qb + 1) * 512)
                ks = ps_s.tile([128, GP, 512], F32, tag="ks", name="ks")
                for j in range(GP):
                    kb = g * GP + j
                    nc.tensor.matmul(
                        ks[:, j, :],
                        lhsT=kT[:, kb * 128:(kb + 1) * 128],
                        rhs=qT[:, h, qsl],
                        start=True,
                        stop=True,
                    )
                return ks

            # flat (unit, group) pipeline: the scores skew carries across
            # unit boundaries so the PE/ACT streams never drain
            flat = [(u, g) for u in range(len(units)) for g in range(NG)]
            po_tiles = {}
            pend = s_group(*flat[0])
            for i, (u, g) in enumerate(flat):
                ks = pend
                pend = s_group(*flat[i + 1]) if i + 1 < len(flat) else None
                if u == 0 and g // 2 + 1 < QB:
                    production(g // 2 + 1, part=g % 2)
                if g == 1:
                    for f in pending_recip:
                        f()
                    pending_recip.clear()
                elif g == 8:
                    for f in pending_bc:
                        f()
                    pending_bc.clear()
                elif g in (9, 11, 13, 15) and pending_proj:
                    pending_proj.pop(0)()
                et = expp.tile([128, GP, 512], FP8)
                if g in SCH_GROUPS:
                    # Schraudolph: e4m3 bits of exp(SCALE*s) via one DVE
                    # affine op, int8-round into the fp8 tile
                    exp_bi = nc.vector.tensor_scalar(
                        out=et[:, :, :].bitcast(I8),
                        in0=ks,
                        scalar1=SCH8_A,
                        scalar2=SCH8_B,
                        op0=ALU.mult,
                        op1=ALU.add,
                    )
                else:
                    exp_bi = nc.scalar.activation(
                        out=et, in_=ks, func=AF.Exp, scale=SCALE
                    )
                last_exp["inst"] = exp_bi.ins
                qb, h = units[u]
                if g == 0:
                    po_tiles[u] = ps_o.tile([HD + 1, 512], F32, name="po")
                po = po_tiles[u]
                # one DoubleRow matmul contracts both key chunks of the
                # group (virtual K=256: slot s pairs v_nat chunk 2g+s with
                # et[:, s, :]) at 2 fp8 MACs/cell/cycle
                nc.tensor.matmul(
                    po,
                    lhsT=v_nat[:, 2 * g:2 * g + 2, h, 0:HD + 1],
                    rhs=et,
                    perf_mode=mybir.MatmulPerfMode.DoubleRow,
                    start=(g == 0),
                    stop=(g == NG - 1),
                )
                if g == NG - 1:
                    qsl = slice(qb * 512, (qb + 1) * 512)
                    recip, bcmult = make_tail(po_tiles.pop(u), h, qsl, u)
                    pending_recip.append(recip)
                    pending_bc.append(bcmult)
                    if h == HPC - 1:
                        for j in range(4):
                            pending_proj.append(make_proj(qb, j))
            for f in pending_recip:
                f()
            for f in pending_bc:
                f()
            for f in pending_proj:
                f()

    return nc


_PROGRAM = None


def _get_program():
    global _PROGRAM
    if _PROGRAM is None:
        _PROGRAM = _build_program()
    return _PROGRAM


def _bf16(a):
    import ml_dtypes

    return np.asarray(a, dtype=np.float32).astype(ml_dtypes.bfloat16)


def _prep_core_inputs(x, W_qkv, b_qkv, heads, batch):
    """Host-side slicing/relayout for one core."""
    cols = np.concatenate([np.arange(h * HD, (h + 1) * HD) for h in heads])
    w_q = W_qkv[:, cols]               # [512, 128]
    w_k = W_qkv[:, C + cols]
    w_v = W_qkv[:, 2 * C + cols]
    # q and k: both heads stacked in one M=128 projection
    w_q = np.ascontiguousarray(w_q.reshape(NCJ, 128, 128).transpose(1, 0, 2))
    w_k = np.ascontiguousarray(w_k.reshape(NCJ, 128, 128).transpose(1, 0, 2))
    # v: [v0 | 0 | v1 | 0] with zero cols where the ones columns go
    wv4 = w_v.reshape(NCJ, 128, HPC, HD)
    w_vp = np.zeros((NCJ, 128, VW), dtype=np.float32)
    w_vp[:, :, 0:HD] = wv4[:, :, 0, :]
    w_vp[:, :, HD + 1:VW - 1] = wv4[:, :, 1, :]
    w_v = np.ascontiguousarray(w_vp.transpose(1, 0, 2))

    # per-head live-row masks and masked biases for the qT split
    qmask = np.zeros((128, HPC), dtype=np.float32)
    b_qm = np.zeros((128, HPC), dtype=np.float32)
    bq = b_qkv[cols]
    for h in range(HPC):
        qmask[h * HD:(h + 1) * HD, h] = 1.0
        b_qm[h * HD:(h + 1) * HD, h] = bq[h * HD:(h + 1) * HD]
    b_k = b_qkv[C + cols].reshape(128, 1).astype(np.float32)
    xt = np.ascontiguousarray(
        x[batch].T.reshape(NCJ, 128, QB, 512).transpose(2, 1, 0, 3))
    return {
        "ones": np.ones((1, HD), dtype=np.float32),
        "xt": _bf16(xt),
        "w_q": _bf16(w_q),
        "w_k": _bf16(w_k),
        "w_v": _bf16(w_v),
        "b_qm": b_qm,
        "qmask": qmask,
        "b_k": b_k,
    }


def _core_w_o(W_out, heads):
    rows = np.concatenate([np.arange(h * HD, (h + 1) * HD) for h in heads])
    return _bf16(W_out[rows])


def kernel(x, W_qkv, b_qkv, W_out, b_out):
    x = np.asarray(x, dtype=np.float32)
    W_qkv = np.asarray(W_qkv, dtype=np.float32)
    b_qkv = np.asarray(b_qkv, dtype=np.float32)
    W_out = np.asarray(W_out, dtype=np.float32)
    b_out = np.asarray(b_out, dtype=np.float32)

    nc = _get_program()
    in_maps = []
    for c in range(NCORES):
        batch, hp = c // 4, c % 4
        heads = [2 * hp, 2 * hp + 1]
        im = _prep_core_inputs(x, W_qkv, b_qkv, heads, batch)
        im["w_o"] = _core_w_o(W_out, heads)
        in_maps.append(im)

    res = run_bass_kernel_spmd(nc, in_maps, core_ids=list(range(NCORES)))

    # v-bias commutes: softmax rows sum to 1, so (P @ (V + 1 b_v)) @ W_o
    # = P@V@W_o + b_v@W_o. Add b_v@W_out and b_out once on the host.
    const_row = b_qkv[2 * C:] @ W_out + b_out    # [512]
    out = np.empty((B, N, C), dtype=np.float32)
    for b in range(B):
        acc = res.results[4 * b]["out"].astype(np.float32).copy()
        for c in range(4 * b + 1, 4 * b + 4):
            acc += res.results[c]["out"]
        out[b] = acc + const_row
    return out


# revision 8
# speedup vs baseline: 1.3345x; 1.0912x over previous
"""Multi-head attention (B=2, N=4096, C=512, H=8) on 8 TRN2 NeuronCores.

Sharding: core c handles batch c//4 and heads {2*(c%4), 2*(c%4)+1}
(data parallel over batch, tensor parallel over heads). Each core
computes its 2 heads' attention plus a partial output projection;
the host sums the 4 partials per batch and adds the bias terms
(b_out and b_v @ W_out, which commutes past softmax-weighted sums).

Design (evolved from a bf16 baseline that was dual-roofline-bound at
~84% busy on BOTH the tensor engine and the exp activations):

  scores      kT holds both heads stacked on the contraction dim; qT
              is zero-padded per head (dead 64 rows select the head
              and dodge the K=64/M=128 half-rate path).  Output-rate
              bound at 128 scores/cycle -- unchangeable floor.
  exp         alternates between ACT (even key-groups, true exp) and
              DVE (odd groups, Schraudolph bit-trick: e4m3 bits =
              round(8/ln2 * s + 55.625) via one int8-rounding affine
              op) so the two engines exponentiate concurrently and
              the exp stream never paces the PE pipeline.
  PV          fp8(e4m3) DoubleRow matmuls: one matmul contracts both
              key chunks of a group (virtual K=256, 2 fp8 MACs per
              cell per cycle) -- half the bf16 streaming time.  The
              v operand is produced as [v|1|pad] per head (ones give
              softmax sums for free; 16B-aligned pitch for the
              DoubleRow weight AP).
  q/k/v prod  one stacked M=128 matmul per projection per 512-token
              block; per-head qT halves split out by ACT/DVE
              mask-mult+bias; each production part packs into ONE
              shared-ring PSUM slot to keep unit-0 slot churn low.
  normalize   1/sums broadcast across partitions via a K=1 f32r
              ones-matmul (no DRAM round-trip), ACT-staged to SBUF
              (tensor_tensor allows a single PSUM operand); h1
              writes cross-partition-base so oT stacks both heads.
  out proj    oT holds h0 dims on partitions 0:64 and h1 on 64:128:
              ONE K=128 matmul per 128-token chunk.
  scheduling  3 score-pipeline PSUM slots (tag-shared with
              production/projection/broadcast tiles; exactly 8 banks
              with the two PV accumulators); tails flush at fixed
              group offsets of later units; the closing unit runs
              its tail chain in pipelined 256-column halves and fans
              the final stores across queues.

Accuracy: rel-to-scale 1.52e-2 on the fixed harness inputs (gate
2e-2); the fp8 PV path dominates the error budget and was validated
end-to-end in numpy (1.48e-2 predicted) before adoption.
"""

import numpy as np

import concourse.bass as bass
import concourse.mybir as mybir
import concourse.tile as tile
from concourse.bass_utils import run_bass_kernel_spmd
from concourse.tile_rust import add_dep_helper
from concourse.vector_clock import ScopedClock

F32 = mybir.dt.float32
F32R = mybir.dt.float32r
BF16 = mybir.dt.bfloat16
FP8 = mybir.dt.float8e4
I16 = mybir.dt.int16
I8 = mybir.dt.int8
AF = mybir.ActivationFunctionType
ALU = mybir.AluOpType

B, N, C, H = 2, 4096, 512, 8
HD = C // H          # 64
HPC = H // 4         # 2 heads per core
NCORES = 8
NT = N // 128        # 32 key chunks
NCJ = C // 128       # 4 contraction chunks
QB = N // 512        # 8 query blocks
GP = 2               # key chunks per exp group (PSUM banks per scores tile)
NG = NT // GP
NUNITS = QB * HPC
VW = 2 * (HD + 1)    # v_nat row width: [v0|1][v1|1] (ones via memset)
SCALE = 1.0 / float(np.sqrt(C))
# Schraudolph bf16 exp: bits16 = round(SCH_A * score + SCH_B) viewed as bf16
SCH_A = 128.0 / float(np.log(2.0)) * SCALE
SCH_B = 16250.5
# fp8(e4m3) variant feeding the DoubleRow PV matmul
SCH8_A = 8.0 / float(np.log(2.0)) * SCALE
SCH8_B = 55.625
VP8 = 80             # fp8 v_nat per-head row pitch (16B-aligned for DoubleRow)
# odd groups exponentiate on DVE, even on ACT: the two engines run
# concurrently so the exp stream (~1.1us/group on one engine) stops
# pacing the 854ns/group PE pipeline
SCH_GROUPS = frozenset(range(1, NG, 2))
# one even group per unit rides the otherwise-idle Pool engine (ACT
# stages the PSUM scores to SBUF, the Q7s do the int8 Schraudolph);
# its PV is deferred two groups so the slow producer never
# head-blocks the in-order PE queue
POOL_GROUPS = frozenset()


def _patch_tail_drain():
    """This walrus build caps sync waits at 1 per non-EventSemaphore
    instruction (2 for EventSemaphore); the stock TileContext tail-drain
    attaches every outstanding wait to one Drain, and the scheduler can
    leave >1 wait on regular instructions. Spill extras onto fresh
    same-engine nops inserted just before the over-subscribed one."""
    if getattr(tile.TileContext, "_drain_patched", False):
        return

    def _spill_excess_waits(nc):
        for fn in nc.m.functions:
            for bb in fn.blocks:
                insts = bb.instructions
                i = 0
                while i < len(insts):
                    inst = insts[i]
                    si = inst.sync_info
                    cap = 2 if isinstance(inst, mybir.InstEventSemaphore) else 1
                    if si is None or len(si.on_wait) <= cap:
                        i += 1
                        continue
                    extra = list(si.on_wait[cap:])
                    si.on_wait[:] = si.on_wait[:cap]
                    for w in extra:
                        nop = nc.engines[inst.engine].nop(
                            hint="wait_spill", nofuse=True
                        )
                        cur = nc.cur_bb.bb.instructions
                        cur.remove(nop.ins)
                        if nop.ins.sync_info is None:
                            nop.ins.sync_info = mybir.SyncInfo(
                                on_update=[], on_wait=[]
                            )
                        nop.ins.sync_info.on_wait.append(w)
                        insts.insert(i, nop.ins)
                        i += 1
                    i += 1

    def _drain_and_barrier(self, tick_clock, wait_clock):
        nc = self.nc
        drain_inst = nc.sync.drain()
        wait_clock.add_sem_waits(
            drain_inst.ins, ScopedClock({None: tick_clock.global_clock})
        )
        nc.all_engine_barrier()
        assert self.sems is not None
        popped = nc._tile_sem_poison_stack.pop()
        assert popped is self._sem_poison
        nc.clear_and_free_semaphores(list(self.sems.allocated().values()))
        nc.all_engine_barrier()
        _spill_excess_waits(nc)

    tile.TileContext._drain_and_barrier = _drain_and_barrier
    tile.TileContext._drain_patched = True


def _build_program():
    _patch_tail_drain()
    nc = bass.Bass()

    xt = nc.dram_tensor("xt", [QB, 128, NCJ, 512], BF16, kind="ExternalInput")
    # host-prearranged weight layouts (see kernel() below)
    w_q = nc.dram_tensor("w_q", [128, NCJ, 128], BF16, kind="ExternalInput")
    w_k = nc.dram_tensor("w_k", [128, NCJ, 128], BF16, kind="ExternalInput")
    # v weights carry zero cols at 64 and 129 so one copy + tiny memsets
    # yield the augmented [v0|1][v1|1] PV-operand layout
    w_v = nc.dram_tensor("w_v", [128, NCJ, VW], BF16, kind="ExternalInput")
    w_o = nc.dram_tensor("w_o", [128, C], BF16, kind="ExternalInput")
    b_qm = nc.dram_tensor("b_qm", [128, HPC], F32, kind="ExternalInput")
    qmask = nc.dram_tensor("qmask", [128, HPC], F32, kind="ExternalInput")
    b_k = nc.dram_tensor("b_k", [128, 1], F32, kind="ExternalInput")
    ones = nc.dram_tensor("ones", [1, HD], F32R, kind="ExternalInput")
    out = nc.dram_tensor("out", [N, C], F32, kind="ExternalOutput")

    from contextlib import ExitStack

    with tile.TileContext(nc) as tc, ExitStack() as ctx:
        const = ctx.enter_context(tc.tile_pool(name="const", bufs=1))
        w_q_sb = const.tile([128, NCJ, 128], BF16)
        w_k_sb = const.tile([128, NCJ, 128], BF16)
        w_v_sb = const.tile([128, NCJ, VW], BF16)
        w_o_sb = const.tile([128, C], BF16)
        b_qm_sb = const.tile([128, HPC], F32)
        qmask_sb = const.tile([128, HPC], F32)
        b_k_sb = const.tile([128, 1], F32)
        ones_sb = const.tile([1, HD], F32R)
        # touch ACT immediately so the exp_and_others table load (~1.3us)
        # overlaps the initial weight DMAs instead of the first kT add
        warm = const.tile([1, 1], F32)
        nc.vector.memset(warm, 0.0)
        nc.scalar.activation(out=warm, in_=warm, func=AF.Exp)


        persist = ctx.enter_context(tc.tile_pool(name="persist", bufs=1))
        # K=128 zero-padded: per head, the dead 64 rows stay zero (via the
        # qmask mult) -- K=64 M=128 single-group matmuls run at half rate
        qT = persist.tile([128, HPC, N], BF16)
        # kT holds BOTH heads stacked (rows 0:64 = h0, 64:128 = h1); the
        # per-head qT zero rows select the head in the scores contraction
        kT = persist.tile([128, N], BF16)
        # fp8 [tokens, kb, head, VP8]: [v dims | 1 | pad] per key chunk --
        # lhsT of the DoubleRow PV matmul ([128, 2, 65] per chunk pair)
        v_nat = persist.tile([128, NT, HPC, VP8], FP8)

        # ---- fused pipeline: qkv production interleaved into attention ----
        with (
            tc.tile_pool(name="xTp", bufs=1) as xTp,
            tc.tile_pool(name="oTp", bufs=1) as oTp,
            tc.tile_pool(name="expp", bufs=6) as expp,
            tc.tile_pool(name="recipp", bufs=3) as recipp,
            tc.tile_pool(name="bcsb", bufs=3) as bcsb,
            tc.tile_pool(name="stgp", bufs=2) as stgp,
            tc.tile_pool(name="ostage", bufs=4) as ostage,
            tc.tile_pool(name="ps_s", bufs=3, space="PSUM") as ps_s,
            tc.tile_pool(name="ps_o", bufs=2, space="PSUM") as ps_o,
        ):
            # production/projection PSUM tiles borrow score-pipeline slots
            # (tag-shared, temporally interleaved); 3x[128,2,512] score
            # slots + 2x[65,512] po = exactly 8 PSUM banks
            ps_p = ps_s
            # oT: h0 dims on partitions 0:64, h1 dims on 64:128 -- the
            # out-projection contracts both heads in one K=128 matmul
            oT = oTp.tile([128, N], BF16)
            xT = xTp.tile([128, NCJ, N], BF16)
            # block-0 x and the earliest-consumed weights first, so
            # production(0) isn't blocked behind the full weight load;
            # split per cj across queues so the four DMAs run in parallel
            nc.sync.dma_start(out=w_k_sb, in_=w_k[:])
            for cj, eng in enumerate((nc.scalar, nc.gpsimd, nc.sync,
                                      nc.scalar)):
                eng.dma_start(out=xT[:, cj, 0:512], in_=xt[0, :, cj])
            nc.sync.dma_start(out=b_k_sb, in_=b_k[:])
            nc.sync.dma_start(out=w_v_sb, in_=w_v[:])
            nc.sync.dma_start(out=w_q_sb, in_=w_q[:])
            nc.sync.dma_start(out=b_qm_sb, in_=b_qm[:])
            nc.sync.dma_start(out=qmask_sb, in_=qmask[:])
            nc.sync.dma_start(out=ones_sb, in_=ones[:])
            for tb in range(1, QB):
                tsl = slice(tb * 512, (tb + 1) * 512)
                nc.sync.dma_start(out=xT[:, :, tsl], in_=xt[tb])
            nc.sync.dma_start(out=w_o_sb, in_=w_o[:])

            def production(tb, part=None):
                """qkv projections for one 512-token block. Each part
                packs its k-or-q projection plus two v chunks into ONE
                shared-ring PSUM slot (sub-regions of a [128, 2, 512]
                tile) so unit-0's slot churn stays low.
                part 0: k (scores consume it soonest) + first half of v;
                part 1: rest of v + q. None: both."""
                tsl = slice(tb * 512, (tb + 1) * 512)

                def vnat_pair(pt, kb0):
                    for i in range(2):
                        ksl = slice((kb0 + i) * 128, (kb0 + i + 1) * 128)
                        for cj in range(NCJ):
                            nc.tensor.matmul(
                                pt[:, 1, i * VW:(i + 1) * VW],
                                lhsT=xT[:, cj, ksl],
                                rhs=w_v_sb[:, cj, :],
                                start=(cj == 0),
                                stop=(cj == NCJ - 1),
                            )
                    nc.scalar.activation(
                        out=v_nat[:, kb0:kb0 + 2, :, 0:HD + 1],
                        in_=pt[:, 1, 0:2 * VW].rearrange(
                            "p (kb h d) -> p kb h d", kb=2, h=HPC
                        ),
                        func=AF.Copy,
                    )
                    # restore the softmax-sum ones over the zero cols
                    nc.gpsimd.memset(v_nat[:, kb0:kb0 + 2, :, HD:HD + 1], 1.0)

                if part in (0, None):
                    pt = ps_p.tile([128, 2, 512], F32, tag="ks", name="pt")
                    for cj in range(NCJ):
                        nc.tensor.matmul(
                            pt[:, 0, :],
                            lhsT=w_k_sb[:, cj, :],
                            rhs=xT[:, cj, tsl],
                            start=(cj == 0),
                            stop=(cj == NCJ - 1),
                        )
                    nc.scalar.activation(
                        out=kT[:, tsl], in_=pt[:, 0, :], func=AF.Identity,
                        bias=b_k_sb[:, 0:1],
                    )
                    vnat_pair(pt, tb * 4)
                if part in (1, None):
                    pt = ps_p.tile([128, 2, 512], F32, tag="ks", name="pt")
                    vnat_pair(pt, tb * 4 + 2)

            def q_production(tb):
                """Deferred: block tb's q isn't consumed until unit 2*tb,
                so it runs inside unit 2*tb-1 instead of congesting the
                unit-0 PSUM ring. ONE stacked M=128 matmul; per-head
                halves split out by mask-mult (dead rows -> 0) +
                masked-bias add."""
                tsl = slice(tb * 512, (tb + 1) * 512)
                pq = ps_p.tile([128, 512], F32, tag="ks", name="pq")
                for cj in range(NCJ):
                    nc.tensor.matmul(
                        pq,
                        lhsT=w_q_sb[:, cj, :],
                        rhs=xT[:, cj, tsl],
                        start=(cj == 0),
                        stop=(cj == NCJ - 1),
                    )
                nc.scalar.activation(
                    out=qT[:, 0, tsl], in_=pq, func=AF.Identity,
                    scale=qmask_sb[:, 0:1], bias=b_qm_sb[:, 0:1],
                )
                nc.vector.tensor_scalar(
                    out=qT[:, 1, tsl],
                    in0=pq,
                    scalar1=qmask_sb[:, 1:2],
                    scalar2=b_qm_sb[:, 1:2],
                    op0=ALU.mult,
                    op1=ALU.add,
                )

            production(0)
            q_production(0)
            last_exp = {"inst": None}
            pending_recip = []  # flushed @g1 of the following unit (DVE)
            pending_bc = []     # flushed @g8 (Pool bcast DMA + DVE normalize)
            pending_proj = []   # flushed @g9/11/13/15 (PE matmuls)

            def make_tail(po, h, qsl, u):
                state = {}
                prow = slice(0, HD) if h == 0 else slice(HD, 128)
                # last unit: nothing overlaps the closing
                # recip->broadcast->normalize->projection chain, so run it
                # in pipelined 256-column halves
                nhalf = 2 if u == NUNITS - 1 else 1
                csz = 512 // nhalf

                def recip():
                    rt = recipp.tile([1, 512], F32R, name="rt")
                    with nc.allow_low_precision(
                        reason="f32r reciprocal feeds the f32r "
                        "broadcast matmul (19-bit mantissa ample)"
                    ):
                        for ci in range(nhalf):
                            cs = slice(ci * csz, (ci + 1) * csz)
                            nc.vector.reciprocal(
                                out=rt[:, cs], in_=po[HD:HD + 1, cs]
                            )
                    state["rt"] = rt

                def bcmult():
                    # broadcast 1/sums across 64 partitions with a K=1
                    # ones-matmul (f32r streams 512 cols at full rate) --
                    # cheaper and far lower latency than a DRAM round-trip.
                    # Staged through SBUF: tensor_tensor allows only one
                    # PSUM operand (po), so bc must be SBUF-resident.
                    bc = ps_s.tile([HD, 512], F32, tag="ks", name="bc")
                    bcs = bcsb.tile([HD, 512], F32, name="bcs")
                    for ci in range(nhalf):
                        cs = slice(ci * csz, (ci + 1) * csz)
                        qs = slice(qsl.start + ci * csz,
                                   qsl.start + (ci + 1) * csz)
                        nc.tensor.matmul(
                            bc[:, cs],
                            lhsT=ones_sb,
                            rhs=state["rt"][:, cs],
                            start=True,
                            stop=True,
                        )
                        nc.scalar.activation(
                            out=bcs[:, cs], in_=bc[:, cs], func=AF.Copy
                        )
                        nc.vector.tensor_mul(
                            out=oT[prow, qs], in0=po[0:HD, cs],
                            in1=bcs[:, cs],
                        )
                return recip, bcmult

            def make_proj(qb, j):
                def proj():
                    q0 = qb * 512 + j * 128
                    pp = ps_p.tile([128, C], F32, tag="ks", name="pp")
                    mm = nc.tensor.matmul(
                        pp,
                        lhsT=oT[:, q0:q0 + 128],
                        rhs=w_o_sb,
                        start=True,
                        stop=True,
                    )
                    if last_exp["inst"] is not None:
                        add_dep_helper(
                            mm.ins, last_exp["inst"], sync=False,
                            reason="proj after normalize really done",
                        )
                    ot = ostage.tile([128, C], F32, name="ot")
                    if qb == QB - 1:
                        # drained pipeline: fan the last stores across
                        # engines/queues so they don't serialize
                        if j % 2 == 0:
                            nc.scalar.activation(out=ot, in_=pp, func=AF.Copy)
                        else:
                            nc.vector.tensor_copy(out=ot, in_=pp)
                        (nc.sync, nc.scalar, nc.gpsimd, nc.sync)[j].dma_start(
                            out=out[q0:q0 + 128, :], in_=ot
                        )
                    else:
                        nc.scalar.activation(out=ot, in_=pp, func=AF.Copy)
                        nc.sync.dma_start(out=out[q0:q0 + 128, :], in_=ot)
                return proj

            units = [(qb, h) for qb in range(QB) for h in range(HPC)]

            def s_group(u, g):
                qb, h = units[u]
                qsl = slice(qb * 512, (qb + 1) * 512)
                ks = ps_s.tile([128, GP, 512], F32, tag="ks", name="ks")
                for j in range(GP):
                    kb = g * GP + j
                    nc.tensor.matmul(
                        ks[:, j, :],
                        lhsT=kT[:, kb * 128:(kb + 1) * 128],
                        rhs=qT[:, h, qsl],
                        start=True,
                        stop=True,
                    )
                return ks

            # flat (unit, group) pipeline: the scores skew carries across
            # unit boundaries so the PE/ACT streams never drain
            flat = [(u, g) for u in range(len(units)) for g in range(NG)]
            po_tiles = {}
            pend_pv = {}
            pend = s_group(*flat[0])
            for i, (u, g) in enumerate(flat):
                ks = pend
                pend = s_group(*flat[i + 1]) if i + 1 < len(flat) else None
                if u == 0 and g // 2 + 1 < QB:
                    production(g // 2 + 1, part=g % 2)
                if u % 2 == 1 and u <= 13 and g == 10:
                    q_production((u + 1) // 2)
                if g == 2:
                    for f in pending_recip:
                        f()
                    pending_recip.clear()
                elif g == 8:
                    for f in pending_bc:
                        f()
                    pending_bc.clear()
                elif g in (12, 13, 14, 15) and pending_proj:
                    pending_proj.pop(0)()
                et = expp.tile([128, GP, 512], FP8)
                if g in POOL_GROUPS:
                    stg = stgp.tile([128, GP, 512], F32, name="stg")
                    nc.scalar.activation(out=stg, in_=ks, func=AF.Copy)
                    exp_bi = nc.gpsimd.tensor_scalar(
                        out=et[:, :, :].bitcast(I8),
                        in0=stg,
                        scalar1=SCH8_A,
                        scalar2=SCH8_B,
                        op0=ALU.mult,
                        op1=ALU.add,
                    )
                elif g in SCH_GROUPS:
                    # Schraudolph: e4m3 bits of exp(SCALE*s) via one DVE
                    # affine op, int8-round into the fp8 tile
                    exp_bi = nc.vector.tensor_scalar(
                        out=et[:, :, :].bitcast(I8),
                        in0=ks,
                        scalar1=SCH8_A,
                        scalar2=SCH8_B,
                        op0=ALU.mult,
                        op1=ALU.add,
                    )
                else:
                    exp_bi = nc.scalar.activation(
                        out=et, in_=ks, func=AF.Exp, scale=SCALE
                    )
                last_exp["inst"] = exp_bi.ins
                qb, h = units[u]
                if g == 0:
                    po_tiles[u] = ps_o.tile([HD + 1, 512], F32, name="po")
                po = po_tiles[u]

                # one DoubleRow matmul contracts both key chunks of the
                # group (virtual K=256: slot s pairs v_nat chunk 2g+s with
                # et[:, s, :]) at 2 fp8 MACs/cell/cycle
                def issue_pv(gx, etx, pox=po, hx=h):
                    nc.tensor.matmul(
                        pox,
                        lhsT=v_nat[:, 2 * gx:2 * gx + 2, hx, 0:HD + 1],
                        rhs=etx,
                        perf_mode=mybir.MatmulPerfMode.DoubleRow,
                        start=(gx == 0),
                        stop=(gx == NG - 1),
                    )

                if g in POOL_GROUPS:
                    pend_pv[g] = et
                else:
                    issue_pv(g, et)
                if g - 2 in pend_pv:
                    issue_pv(g - 2, pend_pv.pop(g - 2))
                if g == NG - 1:
                    qsl = slice(qb * 512, (qb + 1) * 512)
                    recip, bcmult = make_tail(po_tiles.pop(u), h, qsl, u)
                    pending_recip.append(recip)
                    pending_bc.append(bcmult)
                    if h == HPC - 1:
                        for j in range(4):
                            pending_proj.append(make_proj(qb, j))
            for f in pending_recip:
                f()
            for f in pending_bc:
                f()
            for f in pending_proj:
                f()

    return nc


_PROGRAM = None


def _get_program():
    global _PROGRAM
    if _PROGRAM is None:
        _PROGRAM = _build_program()
    return _PROGRAM


def _bf16(a):
    import ml_dtypes

    return np.asarray(a, dtype=np.float32).astype(ml_dtypes.bfloat16)


def _prep_core_inputs(x, W_qkv, b_qkv, heads, batch):
    """Host-side slicing/relayout for one core."""
    cols = np.concatenate([np.arange(h * HD, (h + 1) * HD) for h in heads])
    w_q = W_qkv[:, cols]               # [512, 128]
    w_k = W_qkv[:, C + cols]
    w_v = W_qkv[:, 2 * C + cols]
    # q and k: both heads stacked in one M=128 projection
    w_q = np.ascontiguousarray(w_q.reshape(NCJ, 128, 128).transpose(1, 0, 2))
    w_k = np.ascontiguousarray(w_k.reshape(NCJ, 128, 128).transpose(1, 0, 2))
    # v: [v0 | 0 | v1 | 0] with zero cols where the ones columns go
    wv4 = w_v.reshape(NCJ, 128, HPC, HD)
    w_vp = np.zeros((NCJ, 128, VW), dtype=np.float32)
    w_vp[:, :, 0:HD] = wv4[:, :, 0, :]
    w_vp[:, :, HD + 1:VW - 1] = wv4[:, :, 1, :]
    w_v = np.ascontiguousarray(w_vp.transpose(1, 0, 2))

    # per-head live-row masks and masked biases for the qT split
    qmask = np.zeros((128, HPC), dtype=np.float32)
    b_qm = np.zeros((128, HPC), dtype=np.float32)
    bq = b_qkv[cols]
    for h in range(HPC):
        qmask[h * HD:(h + 1) * HD, h] = 1.0
        b_qm[h * HD:(h + 1) * HD, h] = bq[h * HD:(h + 1) * HD]
    b_k = b_qkv[C + cols].reshape(128, 1).astype(np.float32)
    xt = np.ascontiguousarray(
        x[batch].T.reshape(NCJ, 128, QB, 512).transpose(2, 1, 0, 3))
    return {
        "ones": np.ones((1, HD), dtype=np.float32),
        "xt": _bf16(xt),
        "w_q": _bf16(w_q),
        "w_k": _bf16(w_k),
        "w_v": _bf16(w_v),
        "b_qm": b_qm,
        "qmask": qmask,
        "b_k": b_k,
    }


def _core_w_o(W_out, heads):
    rows = np.concatenate([np.arange(h * HD, (h + 1) * HD) for h in heads])
    return _bf16(W_out[rows])


def kernel(x, W_qkv, b_qkv, W_out, b_out):
    x = np.asarray(x, dtype=np.float32)
    W_qkv = np.asarray(W_qkv, dtype=np.float32)
    b_qkv = np.asarray(b_qkv, dtype=np.float32)
    W_out = np.asarray(W_out, dtype=np.float32)
    b_out = np.asarray(b_out, dtype=np.float32)

    nc = _get_program()
    in_maps = []
    for c in range(NCORES):
        batch, hp = c // 4, c % 4
        heads = [2 * hp, 2 * hp + 1]
        im = _prep_core_inputs(x, W_qkv, b_qkv, heads, batch)
        im["w_o"] = _core_w_o(W_out, heads)
        in_maps.append(im)

    res = run_bass_kernel_spmd(nc, in_maps, core_ids=list(range(NCORES)))

    # v-bias commutes: softmax rows sum to 1, so (P @ (V + 1 b_v)) @ W_o
    # = P@V@W_o + b_v@W_o. Add b_v@W_out and b_out once on the host.
    const_row = b_qkv[2 * C:] @ W_out + b_out    # [512]
    out = np.empty((B, N, C), dtype=np.float32)
    for b in range(B):
        acc = res.results[4 * b]["out"].astype(np.float32).copy()
        for c in range(4 * b + 1, 4 * b + 4):
            acc += res.results[c]["out"]
        out[b] = acc + const_row
    return out
